# revision 30
# baseline (speedup 1.0000x reference)
"""MoELoRALinear Trainium2 kernel (8-core data-parallel, Bass/Tile).

Math (per token t, out feature o):
    out[t,o] = x[t,:] @ base_w[o,:] + base_b[o]
             + sum_e softmax_e(x[t,:] @ router_w[e,:]) * SCALE
               * sum_r (x[t,:] @ A[e,r,:]) * B[e,o,r]

Strategy:
  - 8192 tokens sharded 8 ways (1024 tokens/core); weights replicated.
  - Host-side layout prep only (transposes/reshapes so every device DMA is
    per-partition contiguous); all FLOPs run on device.
  - Matmuls in float32r (fp32 bits, fast PE mode), fp32 PSUM accumulate.
  - Per 128-token chunk: Y = x @ [A;router].T (N=36 matmul group), softmax
    over 4 router cols on DVE/ACT, gates*SCALE applied to the rank-32
    projection, PE-transposed to [32,128]; the gated projection plus a
    ones-row (for bias) is one extra K=33 matmul accumulated into the same
    PSUM tile as the 16 base-matmul K-chunks.
"""

import os
import sys

import numpy as np

import concourse.bacc as bacc
import concourse.bass as bass
import concourse.mybir as mybir
from concourse import masks
from concourse.bass_utils import run_bass_kernel_spmd
from concourse.tile import TileContext

SCALE = 16.0 / 8.0  # alpha / r

N_CORES = 8
TOK = 8192  # 4 * 2048 tokens total
TPC = TOK // N_CORES  # tokens per core = 1024
D = 2048  # in features
O = 2048  # out features
E = 4
R = 8
ER = E * R  # 32
J = ER + E  # 36: rank-proj cols + router cols
DC = D // 128  # 16 contraction chunks
OCW = 512  # out-feature chunk width (one PSUM bank)
OC = O // OCW  # 4
TC = TPC // 128  # 8 token chunks per core

F32 = mybir.dt.float32
F32R = mybir.dt.float32r
BF16 = mybir.dt.bfloat16

# Results of the last device run (for test harness inspection).
last_run_info: dict = {}

_cached = None


def _build_program():
    nc = bacc.Bacc()

    xt_d = nc.declare_dram_parameter("xt", [128, DC * TPC], F32R, isOutput=False)
    wt_d = nc.declare_dram_parameter("wt", [OC, 128, DC * OCW], F32R, isOutput=False)
    w1t_d = nc.declare_dram_parameter("w1t", [128, DC * J], F32R, isOutput=False)
    bcat_d = nc.declare_dram_parameter("bcat", [ER + 1, O], BF16, isOutput=False)
    out_d = nc.declare_dram_parameter("out", [OC, TC, 128, OCW], F32, isOutput=True)

    with TileContext(nc) as tc:
        with (
            tc.tile_pool(name="cpool", bufs=1) as cpool,
            tc.tile_pool(name="wpool", bufs=2) as wpool,
            tc.tile_pool(name="spool", bufs=3) as spool,
            tc.tile_pool(name="opool", bufs=6) as opool,
            tc.tile_pool(name="mpsum", bufs=6, space="PSUM") as mpsum,
            tc.tile_pool(name="ypsum", bufs=2, space="PSUM") as ypsum,
        ):
            # DMA arrival order on the sync ring: small tables, x chunks 0-3,
            # wt[0] in 1MB pieces (unlocks the first base-matmul groups early),
            # then the rest of x, then wt[1].
            w1tr = cpool.tile([128, DC * J], F32R)
            nc.sync.dma_start(out=w1tr, in_=w1t_d[:, :])
            bcatr = cpool.tile([ER + 1, O], BF16)
            nc.sync.dma_start(out=bcatr, in_=bcat_d[:, :])
            xtr = cpool.tile([128, DC * TPC], F32R)

            def load_x(dc):
                nc.sync.dma_start(
                    out=xtr[:, dc * TPC : (dc + 1) * TPC],
                    in_=xt_d[:, dc * TPC : (dc + 1) * TPC],
                )

            def load_w(wtile, oc):
                for k in range(4):
                    nc.sync.dma_start(
                        out=wtile[:, k * 4 * OCW : (k + 1) * 4 * OCW],
                        in_=wt_d[oc, :, k * 4 * OCW : (k + 1) * 4 * OCW],
                    )

            # Interleave: 4 x-chunks (2MB), then the matching 1MB wt[0]
            # piece, so base matmuls for the first PSUM-bank wave unlock
            # every ~3MB of arrival instead of after 12MB.
            wts = {0: wpool.tile([128, DC * OCW], F32R, name="wtr0", tag="wtr")}
            for k in range(4):
                for dc in range(4 * k, 4 * k + 4):
                    load_x(dc)
                nc.sync.dma_start(
                    out=wts[0][:, k * 4 * OCW : (k + 1) * 4 * OCW],
                    in_=wt_d[0, :, k * 4 * OCW : (k + 1) * 4 * OCW],
                )
            wts[1] = wpool.tile([128, DC * OCW], F32R, name="wtr1", tag="wtr")
            load_w(wts[1], 1)

            ident = cpool.tile([128, 128], F32)
            masks.make_identity(nc, ident)
            # Gated projection, transposed, + ones row (bias), all 8 t-chunks.
            # bf16: the LoRA term is ~6% of the output, so bf16 noise here is
            # negligible next to the fp32r base matmul.
            vwtr = cpool.tile([ER + 1, TC * 128], BF16)
            nc.vector.memset(vwtr[ER : ER + 1, :], 1.0)

            # --- Router/LoRA-down phase, transposed: YT[j, t] with
            # YT[0:32]=proj, YT[32:36]=logits. 32 wide-N matmuls instead of
            # 128 narrow ones (fp32r matmul cost is LDWEIGHTS-bound, ~equal
            # for N=36 and N=512).
            ytps = [
                ypsum.tile([J, 512], F32, name=f"ytps{th}", tag="yb")
                for th in range(2)
            ]
            # Wave A: base-matmul groups for oc=0, t=0..5 (6 PSUM banks).
            # Emit YT + wave-A matmuls interleaved in 4-chunk blocks matching
            # the DMA arrival order (x block k, then wt[0] piece k), so the
            # PE's in-order stream ladders along the arriving data.
            psA = {
                t: mpsum.tile([128, OCW], F32, name=f"ps0_{t}", tag="ps")
                for t in range(6)
            }
            for k in range(4):
                for dc in range(4 * k, 4 * k + 4):
                    for th in range(2):
                        nc.tensor.matmul(
                            ytps[th],
                            lhsT=w1tr[:, dc * J : (dc + 1) * J],
                            rhs=xtr[:, dc * TPC + th * 512 : dc * TPC + (th + 1) * 512],
                            start=(dc == 0),
                            stop=(dc == DC - 1),
                        )
                for dc in range(4 * k, 4 * k + 4):
                    for t in range(6):
                        nc.tensor.matmul(
                            psA[t],
                            lhsT=xtr[:, dc * TPC + t * 128 : dc * TPC + (t + 1) * 128],
                            rhs=wts[0][:, dc * OCW : (dc + 1) * OCW],
                            start=(dc == 0),
                            stop=False,
                        )
            yt_sb = cpool.tile([J, TPC], F32)
            for th in range(2):
                nc.vector.tensor_copy(yt_sb[:, th * 512 : (th + 1) * 512], ytps[th])

            # Per 128-token chunk: transpose YT slice to [t, j], softmax the
            # 4 router columns, gate-and-scale the 32 projection columns,
            # transpose back, append as bf16 into the fused-accum lhsT.
            for t in range(TC):
                yps = ypsum.tile([128, J], F32, name=f"tps{t}", tag="yb")
                nc.tensor.transpose(
                    yps, yt_sb[:, t * 128 : (t + 1) * 128], ident[0:J, 0:J]
                )
                nmax = spool.tile([128, 1], F32, tag="nmax")
                nc.vector.reduce_max(
                    nmax, yps[:, ER:J], axis=mybir.AxisListType.X, negate=True
                )
                e4 = spool.tile([128, E], F32, tag="e4")
                nc.scalar.activation(
                    e4,
                    yps[:, ER:J],
                    mybir.ActivationFunctionType.Exp,
                    bias=nmax[:, 0:1],
                    scale=1.0,
                )
                ssum = spool.tile([128, 1], F32, tag="ssum")
                nc.vector.reduce_sum(ssum, e4, axis=mybir.AxisListType.X)
                rinv = spool.tile([128, 1], F32, tag="rinv")
                nc.vector.reciprocal(rinv, ssum)
                ge = spool.tile([128, E], F32, tag="ge")
                nc.vector.tensor_scalar(
                    out=ge,
                    in0=e4,
                    scalar1=rinv[:, 0:1],
                    scalar2=SCALE,
                    op0=mybir.AluOpType.mult,
                    op1=mybir.AluOpType.mult,
                )
                vw = spool.tile([128, ER], F32, tag="vw")
                for e in range(E):
                    nc.vector.tensor_scalar_mul(
                        vw[:, e * R : (e + 1) * R],
                        yps[:, e * R : (e + 1) * R],
                        ge[:, e : e + 1],
                    )
                tps = ypsum.tile([ER, 128], F32, name=f"vtps{t}", tag="yb")
                nc.tensor.transpose(tps, vw, ident)
                nc.vector.tensor_copy(vwtr[0:ER, t * 128 : (t + 1) * 128], tps)

            # --- Main phase: base matmul + fused LoRA-up/bias accumulation
            def close_group(ps, oc, t):
                nc.tensor.matmul(
                    ps,
                    lhsT=vwtr[:, t * 128 : (t + 1) * 128],
                    rhs=bcatr[:, oc * OCW : (oc + 1) * OCW],
                    start=False,
                    stop=True,
                )
                ot = opool.tile([128, OCW], F32, tag="ot")
                nc.vector.tensor_copy(ot, ps)
                nc.sync.dma_start(out=out_d[oc, t], in_=ot)

            def full_group(wtr, oc, t):
                ps = mpsum.tile([128, OCW], F32, name=f"ps{oc}_{t}", tag="ps")
                for dc in range(DC):
                    nc.tensor.matmul(
                        ps,
                        lhsT=xtr[:, dc * TPC + t * 128 : dc * TPC + (t + 1) * 128],
                        rhs=wtr[:, dc * OCW : (dc + 1) * OCW],
                        start=(dc == 0),
                        stop=False,
                    )
                close_group(ps, oc, t)

            # close wave A, then the two remaining oc=0 groups
            for t in range(6):
                close_group(psA[t], 0, t)
            for t in range(6, TC):
                full_group(wts[0], 0, t)
            for oc in range(1, OC):
                if oc >= 2:
                    wts[oc] = wpool.tile(
                        [128, DC * OCW], F32R, name=f"wtr{oc}", tag="wtr"
                    )
                    load_w(wts[oc], oc)
                for t in range(TC):
                    full_group(wts[oc], oc, t)

    nc.compile()
    return nc


def _round_fp32r(a):
    """Round fp32 array to fp32r (RNE to 11 mantissa bits), matching
    walrus's fp32_to_fp32r. The PE's fast fp32r matmul mode requires
    operands pre-rounded to this grid."""
    b = np.ascontiguousarray(a, dtype=np.float32).view(np.uint32)
    lsb = (b >> np.uint32(12)) & np.uint32(1)
    r = (b + np.uint32(0x7FF) + lsb) & np.uint32(0xFFFFF000)
    return r.view(np.float32)


def _prep_inputs(x, base_w, base_b, A, B, router_w):
    """Host-side layout prep: build per-partition-contiguous DMA images."""
    x2 = np.ascontiguousarray(x, dtype=np.float32).reshape(TOK, D)
    # xt[core][p, dc*TPC + t] = x2[core*TPC + t, dc*128 + p]
    xv = x2.reshape(N_CORES, TPC, DC, 128)
    xt = np.ascontiguousarray(xv.transpose(0, 3, 2, 1)).reshape(N_CORES, 128, DC * TPC)

    # wt[oc, p, dc*OCW + o] = base_w[oc*OCW + o, dc*128 + p]
    wv = np.ascontiguousarray(base_w, dtype=np.float32).reshape(OC, OCW, DC, 128)
    wt = np.ascontiguousarray(wv.transpose(0, 3, 2, 1)).reshape(OC, 128, DC * OCW)

    # W1 = [A flattened to 32 rows; router_w 4 rows] over D
    W1 = np.concatenate(
        [np.asarray(A, dtype=np.float32).reshape(ER, D), np.asarray(router_w, np.float32)],
        axis=0,
    )  # [36, D]
    w1v = W1.reshape(J, DC, 128)
    w1t = np.ascontiguousarray(w1v.transpose(2, 1, 0)).reshape(128, DC * J)

    # bcat rows 0..31: B[e, o, r] -> [er, o]; row 32: base_b  (bf16)
    import ml_dtypes

    bc = np.concatenate(
        [
            np.asarray(B, dtype=np.float32).transpose(0, 2, 1).reshape(ER, O),
            np.asarray(base_b, dtype=np.float32)[None, :],
        ],
        axis=0,
    ).astype(ml_dtypes.bfloat16)  # [33, O]
    return _round_fp32r(xt), _round_fp32r(wt), _round_fp32r(w1t), bc


def kernel(x, base_w, base_b, A, B, router_w):
    global _cached
    if _cached is None:
        _cached = _build_program()
    nc = _cached

    xt, wt, w1t, bc = _prep_inputs(x, base_w, base_b, A, B, router_w)

    in_maps = [
        {"xt": xt[c], "wt": wt, "w1t": w1t, "bcat": bc} for c in range(N_CORES)
    ]
    core_ids = list(range(N_CORES))

    profile = os.environ.get("KERNEL_PROFILE", "0") == "1"
    res = run_bass_kernel_spmd(nc, in_maps, core_ids, trace=profile)

    last_run_info.clear()
    last_run_info["exec_time_ns"] = res.exec_time_ns
    last_run_info["mean_exec_time_ns"] = res.mean_exec_time_ns
    last_run_info["instructions_and_trace"] = res.instructions_and_trace
    last_run_info["profile_json"] = res.profile_json

    # out[core] shape [OC, TC, 128, OCW] -> tokens x features
    full = np.empty((TOK, O), dtype=np.float32)
    for c in range(N_CORES):
        buf = res.results[c]["out"]  # [OC, TC, 128, OCW]
        full[c * TPC : (c + 1) * TPC] = (
            buf.transpose(1, 2, 0, 3).reshape(TPC, O)
        )
    return full.reshape(4, 2048, 2048)


# revision 31
# speedup vs baseline: 1.0326x; 1.0326x over previous
"""MoELoRALinear Trainium2 kernel (8-core data-parallel, Bass/Tile).

Math (per token t, out feature o):
    out[t,o] = x[t,:] @ base_w[o,:] + base_b[o]
             + sum_e softmax_e(x[t,:] @ router_w[e,:]) * SCALE
               * sum_r (x[t,:] @ A[e,r,:]) * B[e,o,r]

Strategy:
  - 8192 tokens sharded 8 ways (1024 tokens/core); weights replicated.
  - Host-side layout prep only (transposes/reshapes so every device DMA is
    per-partition contiguous); all FLOPs run on device.
  - Matmuls in float32r (fp32 bits, fast PE mode), fp32 PSUM accumulate.
  - Per 128-token chunk: Y = x @ [A;router].T (N=36 matmul group), softmax
    over 4 router cols on DVE/ACT, gates*SCALE applied to the rank-32
    projection, PE-transposed to [32,128]; the gated projection plus a
    ones-row (for bias) is one extra K=33 matmul accumulated into the same
    PSUM tile as the 16 base-matmul K-chunks.
"""

import os
import sys

import numpy as np

import concourse.bacc as bacc
import concourse.bass as bass
import concourse.mybir as mybir
from concourse import masks
from concourse.bass_utils import run_bass_kernel_spmd
from concourse.tile import TileContext

SCALE = 16.0 / 8.0  # alpha / r

N_CORES = 8
TOK = 8192  # 4 * 2048 tokens total
TPC = TOK // N_CORES  # tokens per core = 1024
D = 2048  # in features
O = 2048  # out features
E = 4
R = 8
ER = E * R  # 32
J = ER + E  # 36: rank-proj cols + router cols
DC = D // 128  # 16 contraction chunks
OCW = 512  # out-feature chunk width (one PSUM bank)
OC = O // OCW  # 4
TC = TPC // 128  # 8 token chunks per core

F32 = mybir.dt.float32
F32R = mybir.dt.float32r
BF16 = mybir.dt.bfloat16

# Results of the last device run (for test harness inspection).
last_run_info: dict = {}

_cached = None


def _build_program():
    nc = bacc.Bacc()

    xt_d = nc.declare_dram_parameter("xt", [128, DC * TPC], F32R, isOutput=False)
    wt_d = nc.declare_dram_parameter("wt", [OC, 128, DC * OCW], F32R, isOutput=False)
    w1t_d = nc.declare_dram_parameter("w1t", [128, DC * J], F32R, isOutput=False)
    bcat_d = nc.declare_dram_parameter("bcat", [ER + 1, O], BF16, isOutput=False)
    out_d = nc.declare_dram_parameter("out", [OC, TC, 128, OCW], F32, isOutput=True)

    with TileContext(nc) as tc:
        with (
            tc.tile_pool(name="cpool", bufs=1) as cpool,
            tc.tile_pool(name="wpool", bufs=2) as wpool,
            tc.tile_pool(name="spool", bufs=3) as spool,
            tc.tile_pool(name="opool", bufs=6) as opool,
            tc.tile_pool(name="mpsum", bufs=6, space="PSUM") as mpsum,
            tc.tile_pool(name="ypsum", bufs=2, space="PSUM") as ypsum,
        ):
            # Small tables go on the ACT HWDGE ring so the x-chunk stream on
            # the sync ring starts immediately (each small DMA costs ~2us of
            # fixed latency; serializing them ahead of x wastes ~10us).
            w1tr = cpool.tile([128, DC * J], F32R)
            nc.scalar.dma_start(out=w1tr, in_=w1t_d[:, :])
            bcatr = cpool.tile([ER + 1, O], BF16)
            nc.scalar.dma_start(out=bcatr, in_=bcat_d[:, :])
            xtr = cpool.tile([128, DC * TPC], F32R)

            def load_x(dc):
                nc.sync.dma_start(
                    out=xtr[:, dc * TPC : (dc + 1) * TPC],
                    in_=xt_d[:, dc * TPC : (dc + 1) * TPC],
                )

            def load_w(wtile, oc):
                for k in range(4):
                    nc.sync.dma_start(
                        out=wtile[:, k * 4 * OCW : (k + 1) * 4 * OCW],
                        in_=wt_d[oc, :, k * 4 * OCW : (k + 1) * 4 * OCW],
                    )

            # Interleave: 4 x-chunks (2MB), then the matching 1MB wt[0]
            # piece, so base matmuls for the first PSUM-bank wave unlock
            # every ~3MB of arrival instead of after 12MB.
            wts = {0: wpool.tile([128, DC * OCW], F32R, name="wtr0", tag="wtr")}
            for k in range(4):
                for dc in range(4 * k, 4 * k + 4):
                    load_x(dc)
                nc.sync.dma_start(
                    out=wts[0][:, k * 4 * OCW : (k + 1) * 4 * OCW],
                    in_=wt_d[0, :, k * 4 * OCW : (k + 1) * 4 * OCW],
                )
            wts[1] = wpool.tile([128, DC * OCW], F32R, name="wtr1", tag="wtr")
            load_w(wts[1], 1)

            ident = cpool.tile([128, 128], F32)
            masks.make_identity(nc, ident)
            # Gated projection, transposed, + ones row (bias), all 8 t-chunks.
            # bf16: the LoRA term is ~6% of the output, so bf16 noise here is
            # negligible next to the fp32r base matmul.
            vwtr = cpool.tile([ER + 1, TC * 128], BF16)
            nc.vector.memset(vwtr[ER : ER + 1, :], 1.0)

            # --- Router/LoRA-down phase, transposed: YT[j, t] with
            # YT[0:32]=proj, YT[32:36]=logits. 32 wide-N matmuls instead of
            # 128 narrow ones (fp32r matmul cost is LDWEIGHTS-bound, ~equal
            # for N=36 and N=512).
            ytps = [
                ypsum.tile([J, 512], F32, name=f"ytps{th}", tag="yb")
                for th in range(2)
            ]
            # Wave A: base-matmul groups for oc=0, t=0..5 (6 PSUM banks).
            # Emit YT + wave-A matmuls interleaved in 4-chunk blocks matching
            # the DMA arrival order (x block k, then wt[0] piece k), so the
            # PE's in-order stream ladders along the arriving data.
            psA = {
                t: mpsum.tile([128, OCW], F32, name=f"ps0_{t}", tag="ps")
                for t in range(6)
            }
            for k in range(4):
                for dc in range(4 * k, 4 * k + 4):
                    for th in range(2):
                        nc.tensor.matmul(
                            ytps[th],
                            lhsT=w1tr[:, dc * J : (dc + 1) * J],
                            rhs=xtr[:, dc * TPC + th * 512 : dc * TPC + (th + 1) * 512],
                            start=(dc == 0),
                            stop=(dc == DC - 1),
                        )
                for dc in range(4 * k, 4 * k + 4):
                    for t in range(6):
                        nc.tensor.matmul(
                            psA[t],
                            lhsT=xtr[:, dc * TPC + t * 128 : dc * TPC + (t + 1) * 128],
                            rhs=wts[0][:, dc * OCW : (dc + 1) * OCW],
                            start=(dc == 0),
                            stop=False,
                        )
            yt_sb = cpool.tile([J, TPC], F32)
            for th in range(2):
                nc.vector.tensor_copy(yt_sb[:, th * 512 : (th + 1) * 512], ytps[th])

            # Per 128-token chunk: transpose YT slice to [t, j], softmax the
            # 4 router columns, gate-and-scale the 32 projection columns,
            # transpose back, append as bf16 into the fused-accum lhsT.
            for t in range(TC):
                yps = ypsum.tile([128, J], F32, name=f"tps{t}", tag="yb")
                nc.tensor.transpose(
                    yps, yt_sb[:, t * 128 : (t + 1) * 128], ident[0:J, 0:J]
                )
                nmax = spool.tile([128, 1], F32, tag="nmax")
                nc.vector.reduce_max(
                    nmax, yps[:, ER:J], axis=mybir.AxisListType.X, negate=True
                )
                e4 = spool.tile([128, E], F32, tag="e4")
                nc.scalar.activation(
                    e4,
                    yps[:, ER:J],
                    mybir.ActivationFunctionType.Exp,
                    bias=nmax[:, 0:1],
                    scale=1.0,
                )
                ssum = spool.tile([128, 1], F32, tag="ssum")
                nc.vector.reduce_sum(ssum, e4, axis=mybir.AxisListType.X)
                rinv = spool.tile([128, 1], F32, tag="rinv")
                nc.vector.reciprocal(rinv, ssum)
                ge = spool.tile([128, E], F32, tag="ge")
                nc.vector.tensor_scalar(
                    out=ge,
                    in0=e4,
                    scalar1=rinv[:, 0:1],
                    scalar2=SCALE,
                    op0=mybir.AluOpType.mult,
                    op1=mybir.AluOpType.mult,
                )
                vw = spool.tile([128, ER], F32, tag="vw")
                for e in range(E):
                    nc.vector.tensor_scalar_mul(
                        vw[:, e * R : (e + 1) * R],
                        yps[:, e * R : (e + 1) * R],
                        ge[:, e : e + 1],
                    )
                tps = ypsum.tile([ER, 128], F32, name=f"vtps{t}", tag="yb")
                nc.tensor.transpose(tps, vw, ident)
                nc.vector.tensor_copy(vwtr[0:ER, t * 128 : (t + 1) * 128], tps)

            # --- Main phase: base matmul + fused LoRA-up/bias accumulation
            def close_group(ps, oc, t):
                nc.tensor.matmul(
                    ps,
                    lhsT=vwtr[:, t * 128 : (t + 1) * 128],
                    rhs=bcatr[:, oc * OCW : (oc + 1) * OCW],
                    start=False,
                    stop=True,
                )
                ot = opool.tile([128, OCW], F32, tag="ot")
                nc.vector.tensor_copy(ot, ps)
                nc.sync.dma_start(out=out_d[oc, t], in_=ot)

            def full_group(wtr, oc, t):
                ps = mpsum.tile([128, OCW], F32, name=f"ps{oc}_{t}", tag="ps")
                for dc in range(DC):
                    nc.tensor.matmul(
                        ps,
                        lhsT=xtr[:, dc * TPC + t * 128 : dc * TPC + (t + 1) * 128],
                        rhs=wtr[:, dc * OCW : (dc + 1) * OCW],
                        start=(dc == 0),
                        stop=False,
                    )
                close_group(ps, oc, t)

            # close wave A, then the two remaining oc=0 groups
            for t in range(6):
                close_group(psA[t], 0, t)
            for t in range(6, TC):
                full_group(wts[0], 0, t)
            for oc in range(1, OC):
                if oc >= 2:
                    wts[oc] = wpool.tile(
                        [128, DC * OCW], F32R, name=f"wtr{oc}", tag="wtr"
                    )
                    load_w(wts[oc], oc)
                for t in range(TC):
                    full_group(wts[oc], oc, t)

    nc.compile()
    return nc


def _round_fp32r(a):
    """Round fp32 array to fp32r (RNE to 11 mantissa bits), matching
    walrus's fp32_to_fp32r. The PE's fast fp32r matmul mode requires
    operands pre-rounded to this grid."""
    b = np.ascontiguousarray(a, dtype=np.float32).view(np.uint32)
    lsb = (b >> np.uint32(12)) & np.uint32(1)
    r = (b + np.uint32(0x7FF) + lsb) & np.uint32(0xFFFFF000)
    return r.view(np.float32)


def _prep_inputs(x, base_w, base_b, A, B, router_w):
    """Host-side layout prep: build per-partition-contiguous DMA images."""
    x2 = np.ascontiguousarray(x, dtype=np.float32).reshape(TOK, D)
    # xt[core][p, dc*TPC + t] = x2[core*TPC + t, dc*128 + p]
    xv = x2.reshape(N_CORES, TPC, DC, 128)
    xt = np.ascontiguousarray(xv.transpose(0, 3, 2, 1)).reshape(N_CORES, 128, DC * TPC)

    # wt[oc, p, dc*OCW + o] = base_w[oc*OCW + o, dc*128 + p]
    wv = np.ascontiguousarray(base_w, dtype=np.float32).reshape(OC, OCW, DC, 128)
    wt = np.ascontiguousarray(wv.transpose(0, 3, 2, 1)).reshape(OC, 128, DC * OCW)

    # W1 = [A flattened to 32 rows; router_w 4 rows] over D
    W1 = np.concatenate(
        [np.asarray(A, dtype=np.float32).reshape(ER, D), np.asarray(router_w, np.float32)],
        axis=0,
    )  # [36, D]
    w1v = W1.reshape(J, DC, 128)
    w1t = np.ascontiguousarray(w1v.transpose(2, 1, 0)).reshape(128, DC * J)

    # bcat rows 0..31: B[e, o, r] -> [er, o]; row 32: base_b  (bf16)
    import ml_dtypes

    bc = np.concatenate(
        [
            np.asarray(B, dtype=np.float32).transpose(0, 2, 1).reshape(ER, O),
            np.asarray(base_b, dtype=np.float32)[None, :],
        ],
        axis=0,
    ).astype(ml_dtypes.bfloat16)  # [33, O]
    return _round_fp32r(xt), _round_fp32r(wt), _round_fp32r(w1t), bc


def kernel(x, base_w, base_b, A, B, router_w):
    global _cached
    if _cached is None:
        _cached = _build_program()
    nc = _cached

    xt, wt, w1t, bc = _prep_inputs(x, base_w, base_b, A, B, router_w)

    in_maps = [
        {"xt": xt[c], "wt": wt, "w1t": w1t, "bcat": bc} for c in range(N_CORES)
    ]
    core_ids = list(range(N_CORES))

    profile = os.environ.get("KERNEL_PROFILE", "0") == "1"
    res = run_bass_kernel_spmd(nc, in_maps, core_ids, trace=profile)

    last_run_info.clear()
    last_run_info["exec_time_ns"] = res.exec_time_ns
    last_run_info["mean_exec_time_ns"] = res.mean_exec_time_ns
    last_run_info["instructions_and_trace"] = res.instructions_and_trace
    last_run_info["profile_json"] = res.profile_json

    # out[core] shape [OC, TC, 128, OCW] -> tokens x features
    full = np.empty((TOK, O), dtype=np.float32)
    for c in range(N_CORES):
        buf = res.results[c]["out"]  # [OC, TC, 128, OCW]
        full[c * TPC : (c + 1) * TPC] = (
            buf.transpose(1, 2, 0, 3).reshape(TPC, O)
        )
    return full.reshape(4, 2048, 2048)
